# revision 1
# baseline (speedup 1.0000x reference)
"""Relational-GCN layer (gather + degree-normalized segment-mean + MLP head
with BatchNorm) on 8 Trainium2 NeuronCores.

Strategy (edge-parallel, dst-sharded):
  * Nodes are sharded contiguously across the 8 cores (6272 nodes/core);
    every edge is routed to the core owning its destination node, so the
    segment-sum needs no inter-core collective at all.
  * Per core, edges are sorted by destination and grouped into 64-node
    windows.  The gathered source features (fp16, via dma_gather) are
    scatter-added with a one-hot matmul per 128-edge block, accumulating
    each window in PSUM.  Degree normalization multiplies by a broadcast
    1/deg afterwards.
  * The relation-embedding term is algebraically rewritten through a
    (node x relation) count matrix:  sum_e rel[type_e] == C @ rel_emb,
    so it never touches the edge loop.
  * BatchNorm sits after the first Linear, so W1 folds into the GCN
    weights on the host: y = agg@(Wm@W1) + x@(Ws@W1) + Cn@(rel@Wm@W1).
    Biases drop out of the normalized result analytically.
  * BN statistics are reduced on-chip (ACT accum) and AllReduced (2KB).
"""

import os
import sys

import numpy as np

sys.path.insert(0, "/opt/trn_rl_repo")

import concourse.bacc as bacc  # noqa: E402
import concourse.mybir as mybir  # noqa: E402
import concourse.tile as tile  # noqa: E402
from concourse.bass_utils import run_bass_kernel_spmd  # noqa: E402

F16 = mybir.dt.float16
F32 = mybir.dt.float32
I16 = mybir.dt.int16

N_ENT = 50000
N_EDGE = 600000
FEAT = 128
HID = 256
OUT = 128
RELS = 101
BN_EPS = 1e-5

CORES = 8
W = 64                 # nodes per scatter window
NPC = 6272             # nodes per core (= 98 * 64 = 14 * 448)
NW = NPC // W          # 98 windows per core
CH = 448               # nodes per head chunk
NCH = NPC // CH        # 14 chunks (7 windows each)
CWIN = CH // W         # windows per chunk = 7
LO = 32768             # row split of the gather table (int16 index limit)
HI_ROWS = N_ENT - LO

_compiled = {}
LAST_RESULTS = None    # set by kernel(); test.py reads exec time from here
TRACE = bool(int(os.environ.get("GNN_TRACE", "0")))


def _build(k_lo, k_hi):
    nc = bacc.Bacc("TRN2", target_bir_lowering=False, num_devices=CORES,
                   num_swdge_queues=4)

    xlo = nc.dram_tensor("xlo", [LO, FEAT], F16, kind="ExternalInput")
    xhi = nc.dram_tensor("xhi", [HI_ROWS, FEAT], F16, kind="ExternalInput")
    n_lo = NW * k_lo * 128
    n_hi = NW * k_hi * 128
    idxlo = nc.dram_tensor("idxlo", [128, n_lo // 16], I16, kind="ExternalInput")
    idxhi = nc.dram_tensor("idxhi", [128, n_hi // 16], I16, kind="ExternalInput")
    dstnlo = nc.dram_tensor("dstnlo", [128, NW * k_lo], F16, kind="ExternalInput")
    dstnhi = nc.dram_tensor("dstnhi", [128, NW * k_hi], F16, kind="ExternalInput")
    reciprow = nc.dram_tensor("reciprow", [1, NPC], F16, kind="ExternalInput")
    xt = nc.dram_tensor("xt", [FEAT, NPC], F16, kind="ExternalInput")
    cnt = nc.dram_tensor("cnt", [RELS, NPC], F16, kind="ExternalInput")
    wmw1 = nc.dram_tensor("wmw1", [FEAT, HID], F16, kind="ExternalInput")
    wsw1 = nc.dram_tensor("wsw1", [FEAT, HID], F16, kind="ExternalInput")
    relw = nc.dram_tensor("relw", [RELS, HID], F16, kind="ExternalInput")
    w2 = nc.dram_tensor("w2", [HID, OUT], F16, kind="ExternalInput")
    iotain = nc.dram_tensor("iotain", [128, W], F16, kind="ExternalInput")
    onesin = nc.dram_tensor("onesin", [1, 128], F16, kind="ExternalInput")
    smalls = nc.dram_tensor("smalls", [128, 8], F32, kind="ExternalInput")
    outt = nc.dram_tensor("outt", [OUT, NPC], F32, kind="ExternalOutput")

    eq = mybir.AluOpType.is_equal
    mul = mybir.AluOpType.mult
    sub = mybir.AluOpType.subtract
    add = mybir.AluOpType.add
    AF = mybir.ActivationFunctionType

    with tile.TileContext(nc) as tc:
        with (
            tc.tile_pool(name="sb", bufs=1) as sb,
            tc.tile_pool(name="dbl", bufs=2) as dbl,
            tc.tile_pool(name="ps", bufs=6, space="PSUM") as ps,
            tc.tile_pool(name="dram", bufs=1, space="DRAM") as dram,
        ):
            # ---- static loads ----
            iota_f = sb.tile([128, W], F16)
            nc.sync.dma_start(iota_f[:], iotain[:])
            ones1 = sb.tile([1, 128], F16)
            nc.sync.dma_start(ones1[:], onesin[:])
            sm = sb.tile([128, 8], F32)
            nc.sync.dma_start(sm[:], smalls[:])
            xt_sb = sb.tile([FEAT, NPC], F16)
            nc.sync.dma_start(xt_sb[:], xt[:])
            cn_sb = sb.tile([RELS, NPC], F16)
            nc.sync.dma_start(cn_sb[:], cnt[:])
            wm_sb = sb.tile([FEAT, HID], F16)
            nc.sync.dma_start(wm_sb[:], wmw1[:])
            ws_sb = sb.tile([FEAT, HID], F16)
            nc.sync.dma_start(ws_sb[:], wsw1[:])
            rw_sb = sb.tile([RELS, HID], F16)
            nc.sync.dma_start(rw_sb[:], relw[:])
            w2_sb = [sb.tile([128, OUT], F16, tag=f"w2_{h}", name=f"w2sb{h}")
                     for h in range(2)]
            for h in range(2):
                nc.sync.dma_start(w2_sb[h][:], w2[h * 128:(h + 1) * 128, :])
            rr_sb = sb.tile([1, NPC], F16)
            nc.sync.dma_start(rr_sb[:], reciprow[:])
            il_sb = sb.tile([128, n_lo // 16], I16)
            nc.sync.dma_start(il_sb[:], idxlo[:])
            ih_sb = sb.tile([128, n_hi // 16], I16)
            nc.sync.dma_start(ih_sb[:], idxhi[:])
            dl_sb = sb.tile([128, NW * k_lo], F16)
            nc.sync.dma_start(dl_sb[:], dstnlo[:])
            dh_sb = sb.tile([128, NW * k_hi], F16)
            nc.sync.dma_start(dh_sb[:], dstnhi[:])

            # ---- 1/deg broadcast along partitions (K=1 matmul trick) ----
            rb = []
            for j in range(NCH):
                p = ps.tile([128, CH], F32, tag="ps")
                nc.tensor.matmul(p[:], lhsT=ones1[:],
                                 rhs=rr_sb[:, j * CH:(j + 1) * CH],
                                 start=True, stop=True)
                t = sb.tile([128, CH], F16, tag=f"rb{j}")
                nc.scalar.copy(t[:], p[:])
                rb.append(t)

            # ---- edge scatter: gather chunk -> one-hot -> PSUM windows ----
            aggn = [sb.tile([128, CH], F16, tag=f"aggn{j}", name=f"aggn{j}")
                    for j in range(NCH)]
            bl = CWIN * k_lo      # lo blocks per compute chunk
            bh = CWIN * k_hi
            gmlo = gmhi = None
            for c in range(NCH):
                gc, gh = divmod(c, 2)
                if gh == 0:
                    gmlo = dbl.tile([128, 2 * bl, FEAT], F16, tag="mlo",
                                    name=f"gmlo{gc}")
                    nc.gpsimd.dma_gather(
                        gmlo[:], xlo[:],
                        il_sb[:, gc * (2 * bl * 8):(gc + 1) * (2 * bl * 8)],
                        2 * bl * 128, 2 * bl * 128, FEAT, single_packet=False,
                        queue_num=gc % 4)
                    gmhi = dbl.tile([128, 2 * bh, FEAT], F16, tag="mhi",
                                    name=f"gmhi{gc}")
                    nc.gpsimd.dma_gather(
                        gmhi[:], xhi[:],
                        ih_sb[:, gc * (2 * bh * 8):(gc + 1) * (2 * bh * 8)],
                        2 * bh * 128, 2 * bh * 128, FEAT, single_packet=False,
                        queue_num=(gc + 2) % 4)
                mlo = gmlo[:, gh * bl:(gh + 1) * bl, :]
                mhi = gmhi[:, gh * bh:(gh + 1) * bh, :]
                slo = dbl.tile([128, bl, W], F16, tag="slo")
                nc.vector.tensor_tensor(
                    out=slo[:],
                    in0=iota_f[:].rearrange("p (o w) -> p o w", o=1)
                        .to_broadcast([128, bl, W]),
                    in1=dl_sb[:, c * bl:(c + 1) * bl]
                        .rearrange("p (b o) -> p b o", o=1)
                        .to_broadcast([128, bl, W]),
                    op=eq)
                shi = dbl.tile([128, bh, W], F16, tag="shi")
                nc.vector.tensor_tensor(
                    out=shi[:],
                    in0=iota_f[:].rearrange("p (o w) -> p o w", o=1)
                        .to_broadcast([128, bh, W]),
                    in1=dh_sb[:, c * bh:(c + 1) * bh]
                        .rearrange("p (b o) -> p b o", o=1)
                        .to_broadcast([128, bh, W]),
                    op=eq)
                for wi in range(CWIN):
                    acc = ps.tile([128, W], F32, tag="ps")
                    for k in range(k_lo):
                        nc.tensor.matmul(acc[:],
                                         lhsT=mlo[:, wi * k_lo + k, :],
                                         rhs=slo[:, wi * k_lo + k, :],
                                         start=(k == 0), stop=False)
                    for k in range(k_hi):
                        nc.tensor.matmul(acc[:],
                                         lhsT=mhi[:, wi * k_hi + k, :],
                                         rhs=shi[:, wi * k_hi + k, :],
                                         start=False, stop=(k == k_hi - 1))
                    nc.vector.tensor_tensor(
                        out=aggn[c][:, wi * W:(wi + 1) * W],
                        in0=acc[:], in1=rb[c][:, wi * W:(wi + 1) * W], op=mul)

            # ---- head pass A: y' chunks + BN partial stats ----
            ysb = [[None] * NCH, [None] * NCH]
            sump = [sb.tile([128, NCH], F32, tag=f"sump{h}", name=f"sump{h}")
                    for h in range(2)]
            sqp = [sb.tile([128, NCH], F32, tag=f"sqp{h}", name=f"sqp{h}")
                   for h in range(2)]
            for j in range(NCH):
                cs = slice(j * CH, (j + 1) * CH)
                for h in range(2):
                    hs = slice(h * 128, (h + 1) * 128)
                    yp = ps.tile([128, CH], F32, tag="ps")
                    nc.tensor.matmul(yp[:], lhsT=wm_sb[:, hs], rhs=aggn[j][:],
                                     start=True, stop=False)
                    nc.tensor.matmul(yp[:], lhsT=ws_sb[:, hs], rhs=xt_sb[:, cs],
                                     start=False, stop=False)
                    nc.tensor.matmul(yp[:], lhsT=rw_sb[:, hs], rhs=cn_sb[:, cs],
                                     start=False, stop=True)
                    yt = sb.tile([128, CH], F16, tag=f"y{h}_{j}")
                    nc.scalar.activation(yt[:], yp[:], AF.Copy,
                                         accum_out=sump[h][:, j:j + 1])
                    ysb[h][j] = yt
                    sq = dbl.tile([128, CH], F16, tag=f"sq{h}")
                    nc.scalar.activation(sq[:], yt[:], AF.Square,
                                         accum_out=sqp[h][:, j:j + 1])

            # ---- BN stats AllReduce ----
            st = sb.tile([128, 4], F32)
            for h in range(2):
                nc.vector.tensor_reduce(st[:, h:h + 1], sump[h][:],
                                        axis=mybir.AxisListType.X,
                                        op=add)
                nc.vector.tensor_reduce(st[:, 2 + h:3 + h], sqp[h][:],
                                        axis=mybir.AxisListType.X,
                                        op=add)
            bin_ = dram.tile([128, 4], F32)
            bout = dram.tile([128, 4], F32)
            nc.sync.dma_start(bin_[:], st[:])
            nc.gpsimd.collective_compute(
                "AllReduce", add,
                replica_groups=[list(range(CORES))],
                ins=[bin_.opt()], outs=[bout.opt()])
            ar = sb.tile([128, 4], F32)
            nc.sync.dma_start(ar[:], bout[:])

            # ---- scale/shift:  s = gamma/sqrt(var+eps),  t = beta - mean*s ----
            prm = sb.tile([128, 10], F32)
            inv_n = 1.0 / float(N_ENT)
            nc.vector.tensor_scalar(prm[:, 0:2], ar[:, 0:2], inv_n, None, mul)   # mean'
            nc.vector.tensor_scalar(prm[:, 2:4], ar[:, 2:4], inv_n, None, mul)   # E[y'^2]
            nc.vector.tensor_tensor(prm[:, 4:6], prm[:, 0:2], prm[:, 0:2], mul)  # mean'^2
            nc.vector.tensor_tensor(prm[:, 6:8], prm[:, 2:4], prm[:, 4:6], sub)  # var
            nc.vector.tensor_scalar(prm[:, 6:8], prm[:, 6:8], BN_EPS, None, add)
            sd = sb.tile([128, 2], F32)
            nc.scalar.sqrt(sd[:], prm[:, 6:8])
            rsd = sb.tile([128, 2], F32)
            nc.vector.reciprocal(rsd[:], sd[:])
            sc = sb.tile([128, 2], F32)
            nc.vector.tensor_tensor(sc[:], rsd[:], sm[:, 0:2], mul)              # s
            tmp = sb.tile([128, 2], F32)
            nc.vector.tensor_tensor(tmp[:], prm[:, 0:2], sc[:], mul)             # mean'*s
            tf = sb.tile([128, 2], F32)
            nc.vector.tensor_tensor(tf[:], sm[:, 2:4], tmp[:], sub)              # t

            # ---- head pass B: relu(s*y'+t) @ W2 + b2 ----
            for j in range(NCH):
                op = ps.tile([128, CH], F32, tag="ps")
                for h in range(2):
                    yr = dbl.tile([128, CH], F16, tag=f"yr{h}")
                    nc.scalar.activation(yr[:], ysb[h][j][:], AF.Relu,
                                         bias=tf[:, h:h + 1], scale=sc[:, h:h + 1])
                    nc.tensor.matmul(op[:], lhsT=w2_sb[h][:], rhs=yr[:],
                                     start=(h == 0), stop=(h == 1))
                ost = dbl.tile([OUT, CH], F32, tag="ost")
                nc.scalar.activation(ost[:], op[:],
                                     AF.Identity, bias=sm[:, 4:5], scale=1.0)
                nc.sync.dma_start(outt[:, j * CH:(j + 1) * CH], ost[:])

    nc.compile()
    return nc


def _prep(edge_index, edge_type):
    src = edge_index[0].astype(np.int64)
    dst = edge_index[1].astype(np.int64)
    et = edge_type.astype(np.int64)
    deg = np.bincount(dst, minlength=N_ENT)
    recip32 = (1.0 / np.maximum(deg, 1.0)).astype(np.float32)

    cntm = np.bincount(dst * RELS + et, minlength=N_ENT * RELS)
    cntm = cntm.reshape(N_ENT, RELS).astype(np.float32)
    cn_full = (cntm * recip32[:, None]).T.astype(np.float16)   # [101, N]

    core = dst // NPC
    win = (dst % NPC) // W
    stream = (src >= LO).astype(np.int64)
    key = (core * NW + win) * 2 + stream
    order = np.argsort(key, kind="stable")
    skey = key[order]
    counts = np.bincount(key, minlength=CORES * NW * 2)
    k_lo = int(np.ceil(counts.reshape(-1, 2)[:, 0].max() / 128))
    k_hi = int(np.ceil(counts.reshape(-1, 2)[:, 1].max() / 128))
    starts = np.zeros(CORES * NW * 2, np.int64)
    np.cumsum(counts[:-1], out=starts[1:])
    rank = np.arange(N_EDGE) - starts[skey]
    g_core = skey // (NW * 2)
    rem = skey % (NW * 2)
    g_win = rem // 2
    g_str = rem % 2
    karr = np.where(g_str == 0, k_lo, k_hi)
    slot = g_win * karr * 128 + rank

    n_lo = NW * k_lo * 128
    n_hi = NW * k_hi * 128
    idx_lo = np.zeros((CORES, n_lo), np.int64)
    idx_hi = np.zeros((CORES, n_hi), np.int64)
    dstn_lo = np.full((CORES, n_lo), -1.0, np.float16)
    dstn_hi = np.full((CORES, n_hi), -1.0, np.float16)
    ssrc = src[order]
    sdst = dst[order]
    lo = g_str == 0
    hi = ~lo
    idx_lo[g_core[lo], slot[lo]] = ssrc[lo]
    dstn_lo[g_core[lo], slot[lo]] = (sdst[lo] % W).astype(np.float16)
    idx_hi[g_core[hi], slot[hi]] = ssrc[hi] - LO
    dstn_hi[g_core[hi], slot[hi]] = (sdst[hi] % W).astype(np.float16)

    def wrap_idx(a):
        # element i -> [i % 16, i // 16], replicated over the 8 Q7 cores
        w = a.reshape(-1, 16).T.astype(np.int16)
        return np.tile(w, (8, 1)).copy()

    def devlay(a):
        # element i -> [i % 128, i // 128]
        return np.ascontiguousarray(a.reshape(-1, 128).T)

    per_core = []
    for c in range(CORES):
        per_core.append({
            "idxlo": wrap_idx(idx_lo[c]),
            "idxhi": wrap_idx(idx_hi[c]),
            "dstnlo": devlay(dstn_lo[c]),
            "dstnhi": devlay(dstn_hi[c]),
        })
    return per_core, recip32, cn_full, k_lo, k_hi


def kernel(edge_index, edge_type, initial_features, relation_embeddings,
           W_msg, b_msg, W_self, W1, b1, gamma, beta, W2, b2):
    global LAST_RESULTS
    edge_index = np.asarray(edge_index)
    edge_type = np.asarray(edge_type)
    x = np.asarray(initial_features, dtype=np.float32)

    per_core, recip32, cn_full, k_lo, k_hi = _prep(edge_index, edge_type)

    x16 = x.astype(np.float16)
    xlo_t = np.ascontiguousarray(x16[:LO])
    xhi_t = np.ascontiguousarray(x16[LO:])

    Wm = np.asarray(W_msg, np.float64)
    Ws = np.asarray(W_self, np.float64)
    W1_ = np.asarray(W1, np.float64)
    rel = np.asarray(relation_embeddings, np.float64)
    wmw1 = np.ascontiguousarray((Wm @ W1_).astype(np.float16))
    wsw1 = np.ascontiguousarray((Ws @ W1_).astype(np.float16))
    relw = np.ascontiguousarray((rel @ Wm @ W1_).astype(np.float16))
    w2_16 = np.asarray(W2, np.float16)

    smalls = np.zeros((128, 8), np.float32)
    g = np.asarray(gamma, np.float32)
    b = np.asarray(beta, np.float32)
    smalls[:, 0] = g[:128]
    smalls[:, 1] = g[128:]
    smalls[:, 2] = b[:128]
    smalls[:, 3] = b[128:]
    smalls[:, 4] = np.asarray(b2, np.float32)

    iota_np = np.tile(np.arange(W, dtype=np.float16)[None, :], (128, 1))
    ones_np = np.ones((1, 128), np.float16)

    in_maps = []
    for c in range(CORES):
        lo_n, hi_n = c * NPC, min((c + 1) * NPC, N_ENT)
        nv = hi_n - lo_n
        xt_c = np.zeros((FEAT, NPC), np.float16)
        xt_c[:, :nv] = x16[lo_n:hi_n].T
        cn_c = np.zeros((RELS, NPC), np.float16)
        cn_c[:, :nv] = cn_full[:, lo_n:hi_n]
        rr_c = np.ones((1, NPC), np.float16)
        rr_c[0, :nv] = recip32[lo_n:hi_n].astype(np.float16)
        in_maps.append({
            "xlo": xlo_t, "xhi": xhi_t,
            "xt": xt_c, "cnt": cn_c, "reciprow": rr_c,
            "wmw1": wmw1, "wsw1": wsw1, "relw": relw, "w2": w2_16,
            "iotain": iota_np, "onesin": ones_np, "smalls": smalls,
            **per_core[c],
        })

    if (k_lo, k_hi) not in _compiled:
        _compiled[(k_lo, k_hi)] = _build(k_lo, k_hi)
    nc = _compiled[(k_lo, k_hi)]

    res = run_bass_kernel_spmd(nc, in_maps, list(range(CORES)), trace=TRACE)
    LAST_RESULTS = res

    out = np.concatenate([res.results[c]["outt"].T for c in range(CORES)], axis=0)
    return np.ascontiguousarray(out[:N_ENT]).astype(np.float32)

